# revision 51
# baseline (speedup 1.0000x reference)
"""Trainium2 Bass kernel for nn_Attention_77025943487081.

Sharding: batch (4) data-parallel x 2-way head tensor-parallel over 8 cores.
Core c handles batch c//2 and heads [8*(c%2), 8*(c%2)+8). Each core produces a
partial c_proj output (contribution of its 512 input channels) in bf16; the
host adds the two partials per batch plus the c_proj bias in f32.

The per-head Conv2D projections, cross-head mixes, position projections and
biases are algebraically folded (on host) into dense matrices so the device
only runs plain matmuls.

Q/K projections (fp8 DoubleRow with identity split): the folded matrices are
near-identity (diag ~1, mix ~0.02) - a bimodal distribution fp8 cannot hold.
So the device computes q = x + (M - I)x: the residual (M - I), unimodal at
~0.02, is scaled x32 into fp8's sweet spot and runs as 4 DoubleRow matmuls
over fp8 x chunks (+1 bf16 position matmul, also pre-scaled x32); the
psum->bf16 finish op is a scalar_tensor_tensor (psum/32 + x_bf16_chunk)
that adds the exact identity part. The position+bias chunk rides the DR
chain too: it is duplicated into both slots of a 5th chunk-pair at half
weight, so a q/k chain is 5 DoubleRow matmuls, nothing else. Hidden chunks
are rotated per-core so each q/k psum tile's identity chunk sits at a fixed
index. Q's 1/sqrt(D) scale moves into the exp input scale (exp(s/8 - 5)).
The V projection stays all-bf16 (9-chunk chains): V has no cheap identity
path. Per-block x loads are single multi-chunk DMAs and output tiles write
in one DMA each - the HWDGE costs ~625ns of serial time per DMA, so DMA
count is a first-class resource.

Scores stay bf16 (fp8 q.k was measured at ~1.8e-2 output error - too close
to the gate). Off-diagonal probabilities are fp8 (exp(s-5) keeps them in
range) and their PV runs DoubleRow against the pair-interleaved fp8 V copy;
the softmax denominator rides the PV matmul as output row 64 via an
appended ones-column. Diagonal k-tiles keep bf16 probs (the self-score tail
spans too wide a range for one fp8 scale): per (head, block) they form two
units, P0 = k-tiles (0,1) scored (fp8 DR) at full 512 q-width so one
1024-wide exp covers both slots (the causally dead region computes
real-but-unused scores), P1 = k-tiles (2,3) scored on q in [256,512) only;
affine_select masks each 128-wide triangle in place and PV reads only the
causally live columns (bf16, per-tile).

Schedule per 512-seq block: as the attention units of block ss stream
through PE (scores -> exp -> mask -> PV), filler generators pull the next
block's projections (or, on the last block, the deferred c_proj tiles)
between units; scores of unit i+1 are emitted before PV of unit i across
head boundaries. PSUM: 2 proj/c_proj + 4 scores + 2 PV banks, persistent.
"""

import numpy as np
import ml_dtypes
from contextlib import ExitStack

import concourse.bass as bass
import concourse.tile as tile
from concourse import bacc, mybir
from concourse.bass_utils import run_bass_kernel_spmd

F32 = mybir.dt.float32
BF16 = mybir.dt.bfloat16
FP8 = mybir.dt.float8e4
EXP_SHIFT = -5.0   # exp(s-5): keeps fp8e4m3 probabilities in range
RSCALE = 32.0      # fp8 residual (M - I) pre-scale

B, S, E, H, D, P = 4, 2048, 1024, 16, 64, 64
G = 8            # heads per core
NC = 8           # cores
QKD = G * D      # 512 = per-core q (or k) width
NT = S // 128    # 16 seq tiles
ACT_EXP = mybir.ActivationFunctionType.Exp

FP8_PROJ = True  # identity-split fp8 DoubleRow q/k projection chains
HC = 8           # hidden contraction chunks of 128


def build_nc(fp8_proj=FP8_PROJ):
    nc = bacc.Bacc("TRN2", target_bir_lowering=False, debug=False, num_devices=NC)
    nxc = 4 if fp8_proj else HC  # bf16 x chunks kept (identity adds only)
    xT = nc.dram_tensor("xT", [nxc, 128, S], BF16, kind="ExternalInput").ap()
    qdt = FP8 if fp8_proj else BF16
    if not fp8_proj:
        xTp = nc.dram_tensor("xTp", [128, S], BF16, kind="ExternalInput").ap()
        mqkp = nc.dram_tensor("Mqkp", [128, 2 * QKD], BF16, kind="ExternalInput").ap()
        mvp = nc.dram_tensor("Mvp", [128, QKD], BF16, kind="ExternalInput").ap()
    qkc = HC + 2 if fp8_proj else HC
    if fp8_proj:
        xT8 = nc.dram_tensor("xT8", [HC, 128, S], FP8, kind="ExternalInput").ap()
        xTp8 = nc.dram_tensor("xTp8", [128, S], FP8, kind="ExternalInput").ap()
    mqk = nc.dram_tensor("Mqk", [qkc, 128, 2 * QKD], qdt, kind="ExternalInput").ap()
    mv = nc.dram_tensor("Mv", [qkc, 128, QKD], qdt, kind="ExternalInput").ap()
    wc = nc.dram_tensor("Wc", [4, 128, E], BF16, kind="ExternalInput").ap()
    out = nc.dram_tensor("out", [S, E], BF16, kind="ExternalOutput").ap()

    sc_exp = 1.0 / 8.0 if fp8_proj else 1.0

    with nc.allow_low_precision("fp8/bf16 attention datapath"), \
         tile.TileContext(nc) as tc, ExitStack() as top:
        w_p = top.enter_context(tc.tile_pool(name="weights", bufs=1))
        xt_p = top.enter_context(tc.tile_pool(name="xt", bufs=3))
        x8_p = top.enter_context(tc.tile_pool(name="x8", bufs=3))
        xp_p = top.enter_context(tc.tile_pool(name="xp", bufs=2))
        qk_p = top.enter_context(tc.tile_pool(name="qkt", bufs=1))
        va_p = top.enter_context(tc.tile_pool(name="vaug", bufs=1))
        oT_p = top.enter_context(tc.tile_pool(name="oTt", bufs=4))
        pt_p = top.enter_context(tc.tile_pool(name="ptile", bufs=4))
        rc_p = top.enter_context(tc.tile_pool(name="rcp", bufs=6))
        bc_p = top.enter_context(tc.tile_pool(name="bcst", bufs=6))
        ost_p = top.enter_context(tc.tile_pool(name="ost", bufs=4))

        mqk_sb = w_p.tile([128, qkc, 2 * QKD], qdt)
        mv_sb = w_p.tile([128, qkc, QKD], qdt)
        if not fp8_proj:
            mqkp_sb = w_p.tile([128, 2 * QKD], BF16)
            mvp_sb = w_p.tile([128, QKD], BF16)
        wc_sb = w_p.tile([128, 4, E], BF16)
        qkt = [qk_p.tile([128, S], BF16, name=f"qkt{m}") for m in range(8)]
        # fp8 v in (k-tile-pair, parity) layout for DoubleRow PV, plus a
        # bf16 copy for the diagonal tiles (their probs must stay bf16)
        v8 = va_p.tile([128, NT // 2, G, 2, 80], FP8)
        v_aug = va_p.tile([128, NT, G, D + 1], BF16)

        eshift = w_p.tile([128, 1], F32)
        ident = w_p.tile([128, 128], BF16)
        from concourse.masks import make_identity
        make_identity(nc, ident)
        vt_p = top.enter_context(tc.tile_pool(name="vt", bufs=2))
        nc.vector.memset(v8[:, :, :, :, D:D + 1], 1.0)
        nc.vector.memset(v_aug[:, :, :, D:D + 1], 1.0)
        nc.vector.memset(eshift[:, :], EXP_SHIFT)

        # startup DMA, interleaved so the first contraction chunks land first
        xts = [None] * 4
        x8s = [None] * 4
        xps = [None] * 4
        xts[0] = xt_p.tile([128, nxc, 512], BF16, tag="xt", name="xt0")
        if fp8_proj:
            x8s[0] = x8_p.tile([128, qkc, 512], FP8, tag="x8", name="x80")
            # startup stays chunk-pair-grained so the first chains start
            # early: DR step j of every chain needs (mqk, x8) chunks 2j, 2j+1
            nc.sync.dma_start(out=mqk_sb[:, 0, :], in_=mqk[0])
            nc.sync.dma_start(out=x8s[0][:, 0, :], in_=xT8[0][:, 0:512])
            nc.sync.dma_start(out=mqk_sb[:, 1, :], in_=mqk[1])
            nc.sync.dma_start(out=x8s[0][:, 1, :], in_=xT8[1][:, 0:512])
            # pos chunk-pair early: it gates step 4 of every chain
            nc.sync.dma_start(out=mqk_sb[:, HC:qkc, :],
                              in_=mqk[HC:qkc].rearrange("e p s -> p e s"))
            for ec in (HC, HC + 1):
                nc.sync.dma_start(out=x8s[0][:, ec, :], in_=xTp8[:, 0:512])
            nc.sync.dma_start(out=xts[0][:, 0:1, :], in_=xT[0][:, 0:512])
            mvr = mv.rearrange("e p s -> p e s")
            for j in range(1, 4):
                nc.sync.dma_start(
                    out=mv_sb[:, 2 * (j - 1):2 * j, :],
                    in_=mvr[:, 2 * (j - 1):2 * j, :])
                nc.sync.dma_start(
                    out=mqk_sb[:, 2 * j:2 * j + 2, :],
                    in_=mqk[2 * j:2 * j + 2].rearrange("e p s -> p e s"))
                nc.sync.dma_start(
                    out=x8s[0][:, 2 * j:2 * j + 2, :],
                    in_=xT8.rearrange("e p s -> p e s")[:, 2 * j:2 * j + 2, 0:512])
            nc.sync.dma_start(out=mv_sb[:, 6:qkc, :], in_=mvr[:, 6:qkc, :])
        # identity-add chunks land before the finishes need them
        if fp8_proj:
            nc.sync.dma_start(out=xts[0][:, 1:nxc, :],
                              in_=xT.rearrange("e p s -> p e s")[:, 1:nxc, 0:512])
        else:
            nc.sync.dma_start(out=xts[0][:, :, :],
                              in_=xT.rearrange("e p s -> p e s")[:, :, 0:512])
            nc.sync.dma_start(out=mv_sb[:, :, :],
                              in_=mv.rearrange("e p s -> p e s"))
            nc.sync.dma_start(out=mvp_sb[:, :], in_=mvp)
        nc.sync.dma_start(out=wc_sb[:, :, :], in_=wc.rearrange("e p s -> p e s"))

        if fp8_proj:
            # q/k chain: 5 DoubleRow fp8 matmuls (4 hidden residual pairs +
            # the position/bias chunk duplicated into both slots of pair 4)
            def qk_mm(ps, m, ss, step):
                nc.tensor.matmul(
                    ps[:, :],
                    mqk_sb[:, 2 * step:2 * step + 2, m * 128:(m + 1) * 128],
                    x8s[ss][:, 2 * step:2 * step + 2, :],
                    start=(step == 0), stop=(step == 4),
                    perf_mode=mybir.MatmulPerfMode.DoubleRow,
                    skip_group_check=True)

            QK_STEP = 5
        else:
            def qk_mm(ps, m, ss, step):
                if step < HC:
                    nc.tensor.matmul(
                        ps[:, :], mqk_sb[:, step, m * 128:(m + 1) * 128],
                        xts[ss][:, step, :], start=(step == 0), stop=False,
                        skip_group_check=True)
                else:
                    nc.tensor.matmul(
                        ps[:, :], mqkp_sb[:, m * 128:(m + 1) * 128], xps[ss],
                        start=False, stop=True, skip_group_check=True)

            QK_STEP = 9

        if fp8_proj:
            # vT chain: [dim, s] orientation, 5 DoubleRow fp8 matmuls, same
            # identity-split as q/k; PE transposes flip back to [s, dim]
            def v_mm(ps, dt_, ss, step):
                nc.tensor.matmul(
                    ps[:, :],
                    mv_sb[:, 2 * step:2 * step + 2, dt_ * 128:(dt_ + 1) * 128],
                    x8s[ss][:, 2 * step:2 * step + 2, :],
                    start=(step == 0), stop=(step == 4),
                    perf_mode=mybir.MatmulPerfMode.DoubleRow,
                    skip_group_check=True)

            V_STEP = 5
        else:
            def v_mm(ps, sti, ss, step):
                if step < HC:
                    nc.tensor.matmul(
                        ps[:, :], xts[ss][:, step, sti * 128:(sti + 1) * 128],
                        mv_sb[:, step, :], start=(step == 0), stop=False,
                        skip_group_check=True)
                else:
                    nc.tensor.matmul(
                        ps[:, :], xps[ss][:, sti * 128:(sti + 1) * 128],
                        mvp_sb[:, :], start=False, stop=True,
                        skip_group_check=True)

            V_STEP = 9

        # persistent PSUM pools: 2 + 4 + 2 = 8 banks
        pp = top.enter_context(tc.tile_pool(name="pp", bufs=2, space="PSUM"))
        stp_p = top.enter_context(tc.tile_pool(name="stp", bufs=2, space="PSUM"))
        po_p = top.enter_context(tc.tile_pool(name="po", bufs=2, space="PSUM"))

        oTs = [None] * 4

        def finish_qk(m, ss, ps, eng=None):
            """psum q/k residual -> bf16 q/k tile with exact identity add."""
            sl = slice(ss * 512, ss * 512 + 512)
            if eng is None:
                eng = nc.vector
            if fp8_proj:
                eng.scalar_tensor_tensor(
                    out=qkt[m][:, sl], in0=ps[:, :], scalar=1.0 / RSCALE,
                    in1=xts[ss][:, m % 4, :],
                    op0=mybir.AluOpType.mult, op1=mybir.AluOpType.add)
            else:
                eng.tensor_copy(qkt[m][:, sl], ps[:, :])

        def ph1_qk_wave(ss, w):
            ma, mb = w, 4 + w
            pa = pp.tile([128, 512], F32, tag="pp", name=f"qk{ss}w{w}a")
            pb = pp.tile([128, 512], F32, tag="pp", name=f"qk{ss}w{w}b")
            for step in range(QK_STEP):
                qk_mm(pa, ma, ss, step)
                qk_mm(pb, mb, ss, step)
                if step != QK_STEP - 1:
                    yield
            finish_qk(ma, ss, pa)
            finish_qk(mb, ss, pb)
            yield

        def ph1_v_wave(ss, w):
            if not fp8_proj:
                for sti in (2 * w, 2 * w + 1):
                    pv = pp.tile([128, 512], F32, tag="pp", name=f"v{ss}s{sti}")
                    for step in range(V_STEP):
                        v_mm(pv, sti, ss, step)
                        if step % 3 == 2 and step != V_STEP - 1:
                            yield
                    stt = 4 * ss + sti
                    nc.vector.tensor_copy(
                        v8[:, stt // 2, :, stt % 2, 0:D],
                        pv[:, :].rearrange("p (g d) -> p g d", g=G))
                    nc.vector.tensor_copy(
                        v_aug[:, stt, :, 0:D],
                        pv[:, :].rearrange("p (g d) -> p g d", g=G))
                    yield
                return
            for dt_ in (2 * w, 2 * w + 1):
                pv = pp.tile([128, 512], F32, tag="pp", name=f"v{ss}t{dt_}")
                for step in range(V_STEP):
                    v_mm(pv, dt_, ss, step)
                    if step != V_STEP - 1:
                        yield
                vt = vt_p.tile([128, 512], BF16, tag="vt", name=f"vt{ss}t{dt_}")
                nc.vector.scalar_tensor_tensor(
                    out=vt[:, :], in0=pv[:, :], scalar=1.0 / RSCALE,
                    in1=xts[ss][:, dt_, :],
                    op0=mybir.AluOpType.mult, op1=mybir.AluOpType.add)
                ptb = pp.tile([128, 512], BF16, tag="pp", name=f"vb{ss}t{dt_}")
                for k in range(4):
                    nc.tensor.transpose(
                        ptb[:, k * 128:(k + 1) * 128],
                        vt[:, k * 128:(k + 1) * 128], ident[:, :])
                yield
                # grouped copies: one per destination, covering this
                # dim-tile's two heads across the block's four seq tiles
                nc.vector.tensor_copy(
                    v_aug[:, 4 * ss:4 * ss + 4, 2 * dt_:2 * dt_ + 2, 0:D],
                    ptb[:, :].rearrange("p (k g d) -> p k g d", k=4, g=2))
                nc.vector.tensor_copy(
                    v8[:, 2 * ss:2 * ss + 2, 2 * dt_:2 * dt_ + 2, :, 0:D],
                    ptb[:, :].rearrange("p (l r g d) -> p l g r d", l=2, r=2, g=2))
                yield

        QK_YIELDS = QK_STEP
        if fp8_proj:
            V_YIELDS = 2 * (V_STEP + 1)
        else:
            V_YIELDS = 2 * ((V_STEP - 1) // 3 + 1)

        def ph3_tile(ss, qb, pool=None, ptag="pp", act_copy=False):
            """Partial c_proj for seq tile 4*ss+qb; yields every matmul."""
            if pool is None:
                pool = pp
            oT = oTs[ss]
            stt = 4 * ss + qb
            ost = ost_p.tile([128, E], BF16, tag="ost", name=f"ost{ss}q{qb}")
            for half in range(2):
                pc = pool.tile([128, 512], F32, tag=ptag,
                               name=f"pc{ss}q{qb}{half}")
                for hdb in range(4):
                    nc.tensor.matmul(
                        pc[:, :], oT[:, hdb, qb * 128:qb * 128 + 128],
                        wc_sb[:, hdb, half * 512:half * 512 + 512],
                        start=(hdb == 0), stop=(hdb == 3),
                        skip_group_check=True)
                    if half == 0 or hdb != 3:
                        yield
                dst = ost[:, half * 512:half * 512 + 512]
                if act_copy:
                    nc.scalar.activation(dst, pc[:, :],
                                         mybir.ActivationFunctionType.Copy)
                else:
                    nc.vector.tensor_copy(dst, pc[:, :])
            nc.sync.dma_start(out=out[stt * 128:(stt + 1) * 128, :],
                              in_=ost[:, :])
            yield

        DR = mybir.MatmulPerfMode.DoubleRow

        def attn_head(ss, h):
            m, half = h // 2, h % 2
            qt = qkt[m][64 * half:64 * half + 64, :]
            kt = qkt[4 + m][64 * half:64 * half + 64, :]
            po = po_p.tile([65, 512], F32, tag="po", name=f"po{ss}h{h}")
            blk0 = 512 * ss
            state = {"first_pv": True}

            def score_mm(stp_ap, kc, qlo, wd):
                nc.tensor.matmul(
                    stp_ap, kt[:, 128 * kc:128 * kc + 128],
                    qt[:, qlo:qlo + wd],
                    start=True, stop=True, skip_group_check=True)

            def pv_mm(j, rhs, col0, ncol):
                nc.tensor.matmul(
                    po[:, col0:col0 + ncol], v8[:, j, h, :, 0:D + 1], rhs,
                    start=state["first_pv"], stop=False, perf_mode=DR,
                    skip_group_check=True)
                state["first_pv"] = False

            def pv_bf16(kc, rhs, col0, ncol):
                nc.tensor.matmul(
                    po[:, col0:col0 + ncol], v_aug[:, kc, h, :], rhs,
                    start=state["first_pv"], stop=False,
                    skip_group_check=True)
                state["first_pv"] = False

            def emit_scores_pair(j):
                stp = stp_p.tile([128, 1024], F32, tag="stp", name=f"stp{ss}h{h}")
                pt8 = pt_p.tile([128, 2, 512], FP8, tag="pt8", name=f"p8{ss}h{h}")
                for par in range(2):
                    score_mm(stp[:, 512 * par:512 * par + 512],
                             2 * j + par, blk0, 512)
                nc.scalar.activation(
                    pt8[:, :, :].rearrange("p a b -> p (a b)"),
                    stp[:, 0:1024], ACT_EXP, bias=eshift[:, :], scale=sc_exp)
                return pt8

            def emit_pv_pair(sc):
                pt8, j = sc
                pv_mm(j, pt8[:, :, :], 0, 512)

            def diag_mask(pt, sl, c0):
                nc.gpsimd.affine_select(
                    out=pt[:, sl, c0:c0 + 128], in_=pt[:, sl, c0:c0 + 128],
                    compare_op=mybir.AluOpType.is_ge,
                    fill=0.0, base=0, pattern=[[1, 128]],
                    channel_multiplier=-1)

            def tri_mask(pt, c0):
                nc.gpsimd.affine_select(
                    out=pt[:, c0:c0 + 128], in_=pt[:, c0:c0 + 128],
                    compare_op=mybir.AluOpType.is_ge,
                    fill=0.0, base=0, pattern=[[1, 128]],
                    channel_multiplier=-1)

            def emit_scores_d0(_):
                # diag k-tiles (4ss, 4ss+1) packed at their causally valid
                # q-offsets into one flat tile, so a single 896-wide exp
                # covers both; probs stay bf16, triangles masked in place
                stp = stp_p.tile([128, 1024], F32, tag="stp", name=f"sd0{ss}h{h}")
                pt = pt_p.tile([128, 896], BF16, tag="pt16", name=f"d0{ss}h{h}")
                score_mm(stp[:, 0:512], 4 * ss, blk0, 512)
                score_mm(stp[:, 512:896], 4 * ss + 1, blk0 + 128, 384)
                nc.scalar.activation(
                    pt[:, :], stp[:, 0:896], ACT_EXP,
                    bias=eshift[:, :], scale=sc_exp)
                tri_mask(pt, 0)
                tri_mask(pt, 512)
                return pt

            def emit_pv_d0(sc):
                pv_bf16(4 * ss, sc[:, 0:512], 0, 512)
                pv_bf16(4 * ss + 1, sc[:, 512:896], 128, 384)

            def emit_scores_d1(_):
                # diag k-tiles (4ss+2, 4ss+3) likewise: one 384-wide exp
                stp = stp_p.tile([128, 1024], F32, tag="stp", name=f"sd1{ss}h{h}")
                pt = pt_p.tile([128, 896], BF16, tag="pt16", name=f"d1{ss}h{h}")
                score_mm(stp[:, 0:256], 4 * ss + 2, blk0 + 256, 256)
                score_mm(stp[:, 256:384], 4 * ss + 3, blk0 + 384, 128)
                nc.scalar.activation(
                    pt[:, 0:384], stp[:, 0:384], ACT_EXP,
                    bias=eshift[:, :], scale=sc_exp)
                tri_mask(pt, 0)
                tri_mask(pt, 256)
                return pt

            def emit_pv_d1(sc):
                pv_bf16(4 * ss + 2, sc[:, 0:256], 256, 256)
                pv_bf16(4 * ss + 3, sc[:, 256:384], 384, 128)

            def normalize():
                # copy po out first so its PSUM bank frees after one DVE op
                # instead of after the whole recip/broadcast/mul chain
                pos = rc_p.tile([65, 512], F32, tag="pos", name=f"pos{ss}h{h}")
                nc.vector.tensor_copy(pos[:, :], po[:, :])
                # oT[hd, q] = po[d, q] * (1 / po[64, q])
                rcp = rc_p.tile([1, 512], F32, tag="rc", name=f"rcp{ss}h{h}")
                nc.vector.reciprocal(rcp, pos[64:65, :])
                bcst = bc_p.tile([64, 512], F32, tag="bc", name=f"bc{ss}h{h}")
                nc.gpsimd.partition_broadcast(bcst[:, :], rcp[:, :])
                if ss == 3 and h == G - 1:
                    for qb in range(4):
                        nc.vector.tensor_mul(
                            oTs[ss][64 * half:64 * half + 64, m,
                                    qb * 128:(qb + 1) * 128],
                            pos[0:64, qb * 128:(qb + 1) * 128],
                            bcst[:, qb * 128:(qb + 1) * 128])
                else:
                    nc.vector.tensor_mul(
                        oTs[ss][64 * half:64 * half + 64, m, :],
                        pos[0:64, :], bcst[:, :])

            units = []
            for j in range(2 * ss):
                units.append((
                    lambda j=j: (emit_scores_pair(j), j),
                    emit_pv_pair,
                ))
            units.append((lambda: emit_scores_d0(None), emit_pv_d0))
            units.append((lambda: emit_scores_d1(None), emit_pv_d1))
            return units, normalize

        # ---- projections for seq block 0: interleave three QK waves
        # (6 psums borrowed across the idle attention pools) so the PE
        # consumes each chunk's 6 matmuls while the next chunk's DMA lands
        p10 = []
        for i, (pool, tg) in enumerate([(pp, "pp"), (pp, "pp"),
                                        (stp_p, "stp"), (stp_p, "stp"),
                                        (po_p, "po"), (po_p, "po")]):
            t10 = pool.tile([128, 512], F32, tag=tg, name=f"p10_{i}")
            p10.append(t10)
        ms10 = [0, 4, 1, 5, 2, 6]
        for step in range(QK_STEP):
            for i in range(6):
                qk_mm(p10[i], ms10[i], 0, step)
        for i in range(6):
            finish_qk(ms10[i], 0, p10[i])
        pro_gens = [ph1_v_wave(0, 0), ph1_v_wave(0, 1), ph1_qk_wave(0, 3)]
        PRO_STEPS = 2 * V_YIELDS + QK_YIELDS

        # ---- main sweep over 512-seq blocks
        for ss in range(4):
            if ss < 3:
                xts[ss + 1] = xt_p.tile([128, nxc, 512], BF16, tag="xt",
                                        name=f"xt{ss + 1}")
                nxt = slice((ss + 1) * 512, (ss + 2) * 512)
                if fp8_proj:
                    x8s[ss + 1] = x8_p.tile([128, qkc, 512], FP8, tag="x8",
                                            name=f"x8{ss + 1}")
                    nc.sync.dma_start(
                        out=x8s[ss + 1][:, 0:HC, :],
                        in_=xT8.rearrange("e p s -> p e s")[:, :, nxt])
                    for ec in (HC, HC + 1):
                        nc.sync.dma_start(out=x8s[ss + 1][:, ec, :],
                                          in_=xTp8[:, nxt])
                nc.sync.dma_start(out=xts[ss + 1][:, :, :],
                                  in_=xT.rearrange("e p s -> p e s")[:, :, nxt])
                if not fp8_proj:
                    xps[ss + 1] = xp_p.tile([128, 512], BF16, tag="xp",
                                            name=f"xp{ss + 1}")
                    nc.sync.dma_start(out=xps[ss + 1][:, :], in_=xTp[:, nxt])
            oTs[ss] = oT_p.tile([128, 4, 512], BF16, tag="oT", name=f"oT{ss}")

            # PE filler between attention units: blocks 0-2 pull the next
            # block's projections, the last block pulls the deferred c_proj
            gen_list = []
            n_steps_total = 0
            if ss == 0:
                # leftover prologue: block 0's own v waves and last q/k wave
                # run as filler inside block 0's attention (deps via sems)
                gen_list.extend(pro_gens)
                n_steps_total += PRO_STEPS
            if ss < 3:
                for w in range(4):
                    gen_list.append(ph1_qk_wave(ss + 1, w))
                for w in range(2):
                    gen_list.append(ph1_v_wave(ss + 1, w))
                n_steps_total = (11 * (n_steps_total + 4 * QK_YIELDS
                                       + 2 * V_YIELDS)) // 10
            else:
                for pss in range(3):
                    for qb in range(4):
                        gen_list.append(ph3_tile(pss, qb))
                n_steps_total = 96

            gen_iter = iter(gen_list)
            current = {"g": None}

            def pull_one():
                while True:
                    if current["g"] is None:
                        current["g"] = next(gen_iter, None)
                        if current["g"] is None:
                            return False
                    try:
                        next(current["g"])
                        return True
                    except StopIteration:
                        current["g"] = None

            _g = 2 * ss + 2
            head_lo = 0
            total_groups = (8 - head_lo) * _g

            pull_count = {"n": 0.0, "done": 0}

            def make_pull(active, weight=1.0):
                def pull():
                    if not active:
                        return
                    pull_count["n"] += weight
                    target = int((n_steps_total * pull_count["n"]
                                  + total_groups - 1) // max(total_groups, 1))
                    while pull_count["done"] < target:
                        if not pull_one():
                            return
                        pull_count["done"] += 1
                return pull

            # global software pipeline: scores of unit i+1 are emitted
            # before PV of unit i, across head boundaries, so the PE always
            # has queued work while ACT computes the exp
            from collections import deque
            pends = deque()

            def flush_one():
                p = pends.popleft()
                p[0](p[1])
                if p[2] is not None:
                    p[2]()
                p[3]()

            for h in range(G):
                if ss == 0:
                    # block 0 pulls its own producers (leftover prologue
                    # waves); deps only work in emission order, so force the
                    # producers needed by head h out before its units
                    vy = V_YIELDS // 2
                    need = [vy, vy, 2 * vy, 2 * vy, 3 * vy, 3 * vy,
                            4 * vy + QK_YIELDS, 4 * vy + QK_YIELDS][h]
                    while pull_count["done"] < need and pull_one():
                        pull_count["done"] += 1
                units, normalize = attn_head(ss, h)
                nu = len(units)
                # filler is injected during the exp-dense pair units and
                # skipped on the exp-light diagonal units, so ACT never
                # starves at head boundaries; block 0 has only diag units
                if ss == 0:
                    weights = [1.0] * nu
                else:
                    weights = [nu / (nu - 2.0)] * (nu - 2) + [0.0, 0.0]
                for i, (emit_sc, emit_pv_u) in enumerate(units):
                    pull = make_pull(h >= head_lo, weights[i])
                    sc = emit_sc()
                    # two-deep software pipeline: scores of units u+1 and
                    # u+2 are queued on PE before PV of unit u, so a PV
                    # waiting on its exp/mask never blocks the PE queue head
                    if len(pends) == 7:
                        flush_one()
                    pends.append((emit_pv_u, sc,
                                  normalize if i == len(units) - 1 else None,
                                  pull))
            while pends:
                flush_one()
            while pull_one():
                pass

        # final block's c_proj: attention psum pools are free by now, so
        # the four tiles run round-robin on four banks with copies split
        # across DVE and ACT to shorten the serial tail
        tails = [ph3_tile(3, 0, pp, "pp", False),
                 ph3_tile(3, 1, stp_p, "stp", True),
                 ph3_tile(3, 2, po_p, "po", False),
                 ph3_tile(3, 3, stp_p, "stp", True)]
        while tails:
            for g in list(tails):
                try:
                    next(g)
                except StopIteration:
                    tails.remove(g)

    nc.compile()
    return nc


def prep_core_inputs(hidden_states, position_states, Wq, bq, Wqh, bqh, Wk, bk,
                     Wkh, bkh, Wv, bv, Wvh, bvh, Wp, bp, Wpe, bpe, Wc, bc,
                     fp8_proj=FP8_PROJ):
    """Build the per-core input maps (host-side weight folding + sharding)."""
    bf16 = ml_dtypes.bfloat16
    fp8 = ml_dtypes.float8_e4m3
    f32 = np.float32

    def fused(parity):
        hs = slice(G * parity, G * parity + G)
        # per-core hidden chunk rotation: chunk i := global chunk
        # (4*parity + i) % 8, so q/k psum tile m's identity block sits in
        # chunk m % 4 on every core
        rot = [(4 * parity + i) % HC for i in range(HC)]
        mats = {}
        for name, (Wa, ba, Wh, bh, v) in {
            "q": (Wq, bq, Wqh[hs], bqh[hs], 0),
            "k": (Wk, bk, Wkh[hs], bkh[hs], 1),
            "v": (Wv, bv, Wvh[hs], bvh[hs], 2),
        }.items():
            mx = np.einsum("hed,ghd->hegd", Wa, Wh).reshape(E, QKD)
            mp = np.einsum("pd,g->pgd", Wp[:, v * D:(v + 1) * D], Wpe[v, 0, hs]).reshape(P, QKD)
            bias = (np.einsum("hd,ghd->gd", ba, Wh) + bh
                    + bp[v * D:(v + 1) * D][None, :] * Wpe[v, 0, hs][:, None]
                    + bpe[hs][:, None]).reshape(QKD)
            if not fp8_proj:
                if name == "q":
                    sc = 1.0 / np.sqrt(np.float32(D))
                    mx, mp, bias = mx * sc, mp * sc, bias * sc
                mats[name] = (mx, mp, bias)
            else:
                # identity split: residual (M - I) scaled x32; identity part
                # is added on device from the bf16 x chunk
                mi = mx.copy()
                rows = np.arange(QKD) + G * D * parity  # this core's head rows
                mi[rows, np.arange(QKD)] -= 1.0
                mats[name] = (mi * RSCALE, mp * RSCALE, bias * RSCALE)

        def chunks(mx, mp, bias, dt):
            w = mx.shape[1]
            m8 = mx.reshape(HC, 128, w)[rot]
            mpad = np.zeros((128, w), f32)
            mpad[:P] = mp
            mpad[P] = bias
            if dt is fp8:
                # pos+bias chunk rides the DoubleRow chain: duplicated into
                # both slots of chunk-pair 4 at half weight
                m8 = np.concatenate([m8, (mpad / 2.0)[None], (mpad / 2.0)[None]])
            return np.ascontiguousarray(m8).astype(dt), mpad.astype(bf16)

        qdt = fp8 if fp8_proj else bf16
        q8, qp = chunks(*mats["q"], qdt)
        k8, kp = chunks(*mats["k"], qdt)
        v16, vp = chunks(*mats["v"], qdt)
        mqk8 = np.concatenate([q8, k8], axis=2)
        mqkp = np.concatenate([qp, kp], axis=1)
        wc4 = Wc.reshape(H, D, E)[hs].reshape(QKD, E).reshape(4, 128, E)
        return (np.ascontiguousarray(mqk8), np.ascontiguousarray(mqkp),
                v16, np.ascontiguousarray(vp),
                np.ascontiguousarray(wc4).astype(bf16), rot)

    per_parity = [fused(0), fused(1)]

    in_maps = []
    for c in range(NC):
        b, parity = c // 2, c % 2
        mqk8, mqkp, mv16, mvp, wc4, rot = per_parity[parity]
        xh = np.ascontiguousarray(hidden_states[b].T).reshape(HC, 128, S)[rot]
        xp = np.zeros((128, S), f32)
        xp[:P] = position_states[b].T
        xp[P] = 1.0
        if fp8_proj:
            im = {"xT": np.ascontiguousarray(xh[0:4]).astype(bf16),
                  "Mqk": mqk8, "Mv": mv16, "Wc": wc4,
                  "xT8": np.ascontiguousarray(xh).astype(fp8),
                  "xTp8": xp.astype(fp8)}
        else:
            im = {"xT": np.ascontiguousarray(xh).astype(bf16),
                  "xTp": xp.astype(bf16),
                  "Mqk": mqk8, "Mqkp": mqkp,
                  "Mv": mv16, "Mvp": mvp, "Wc": wc4}
        in_maps.append(im)
    return in_maps


_NC_CACHE = {}


def get_nc():
    if "nc" not in _NC_CACHE:
        _NC_CACHE["nc"] = build_nc()
    return _NC_CACHE["nc"]


def kernel(**inputs):
    nc = get_nc()
    in_maps = prep_core_inputs(**inputs)
    res = run_bass_kernel_spmd(nc, in_maps, list(range(NC)))
    bc = inputs["bc"]
    outs = [res.results[2 * b]["out"].astype(np.float32)
            + res.results[2 * b + 1]["out"].astype(np.float32) + bc
            for b in range(B)]
    return np.stack(outs).astype(np.float32)
